# revision 11
# baseline (speedup 1.0000x reference)
"""Trainium2 Bass kernel for BasisSignalLayer (matmul + 50%-overlap-add).

Reference computation:
    source = einsum("bkn,ln->bkl", weight, basis_signal_weight)   # (B, K, L)
    out    = overlap_and_add(source, L // 2)                       # (B, 32*(K-1)+64)

With L=64 and frame_step=32 (gcd trick in the reference), the scatter-add
reduces to: output subframe j (32 floats) = source[j, 0:32] + source[j-1, 32:64],
with j in [0, K] (source[-1] = source[K] = 0 contributions at the edges).

Per-core dataflow (batch-parallel across 8 cores, one batch element each):
  - DMA natural-layout weight strips (512 frames x 512 basis) into SBUF
  - PE transposes 128x128 tiles to put the contraction dim (n) on partitions
  - ACT/DVE copy transposed tiles PSUM -> SBUF
  - 4 accumulating matmuls: psum(64, F) = basisT.T @ wT  (source.T strip)
  - DVE shifted add: oaa[i, j] = psum[i, j] + psum[32+i, j-1]  (the overlap-add)
  - PE transposes (32,128)->(128,32) to get output rows on partitions
  - DVE copy + DMA out
"""

import numpy as np

import concourse.bacc as bacc
import concourse.bass as bass
import concourse.mybir as mybir
from concourse import tile
from concourse.bass_utils import run_bass_kernel_spmd

FRAMES = 16000
NB = 512  # basis count (contraction dim)
L = 64  # frame length
BATCH = 8
STRIP = 512  # frames per strip
FP32 = mybir.dt.float32


def _strips(frames):
    out, f0 = [], 0
    while f0 < frames:
        F = min(STRIP, frames - f0)
        assert F % 128 == 0
        out.append((f0, F))
        f0 += F
    return out


def build_nc(frames=FRAMES, repeat=1):
    """Build the single-core Bass program (SPMD: same program on all cores)."""
    nc = bacc.Bacc()
    w = nc.dram_tensor("w", [frames, NB], FP32, kind="ExternalInput")
    bT = nc.dram_tensor("bT", [NB, L], FP32, kind="ExternalInput")
    id128 = nc.dram_tensor("id128", [128, 128], FP32, kind="ExternalInput")
    id32 = nc.dram_tensor("id32", [32, 32], FP32, kind="ExternalInput")
    nsub = frames + 1
    out = nc.dram_tensor("out", [nsub, 32], FP32, kind="ExternalOutput")

    strips = _strips(frames)

    with tile.TileContext(nc) as tc:
        with (
            tc.tile_pool(name="consts", bufs=1) as consts,
            tc.tile_pool(name="wn", bufs=3) as wn_pool,
            tc.tile_pool(name="wt", bufs=8) as wt_pool,
            tc.tile_pool(name="oaa", bufs=3) as oaa_pool,
            tc.tile_pool(name="osb", bufs=3) as osb_pool,
            tc.tile_pool(name="ptin", bufs=4, space="PSUM") as ptin_pool,
            tc.tile_pool(name="psrc", bufs=2, space="PSUM") as psrc_pool,
            tc.tile_pool(name="ptout", bufs=1, space="PSUM") as ptout_pool,
        ):
            bT_sb = consts.tile([128, 4 * L], FP32)
            for c in range(4):
                nc.sync.dma_start(
                    out=bT_sb[:, L * c : L * c + L], in_=bT[128 * c : 128 * c + 128, :]
                )
            i128 = consts.tile([128, 128], FP32)
            nc.sync.dma_start(out=i128, in_=id128[:, :])
            i32 = consts.tile([32, 32], FP32)
            nc.sync.dma_start(out=i32, in_=id32[:, :])

            for _rep in range(repeat):
                prev_src = None
                prev_F = None
                for si, (f0, F) in enumerate(strips):
                    q = F // 128
                    # --- load natural strip: (128, q*512), col = qi*512 + n
                    wn = wn_pool.tile([128, (STRIP // 128) * NB], FP32, tag="wn")
                    nc.sync.dma_start(
                        out=wn[:, : q * NB].rearrange("p (q n) -> p q n", n=NB),
                        in_=w[f0 : f0 + F, :].rearrange("(q p) n -> p q n", p=128),
                    )
                    # --- transpose to (n, f) chunks + copy PSUM->SBUF
                    wts = []
                    for c in range(4):
                        tin = ptin_pool.tile([128, STRIP], FP32, tag="ptin")
                        for qi in range(q):
                            nc.tensor.transpose(
                                tin[:, 128 * qi : 128 * qi + 128],
                                wn[:, qi * NB + 128 * c : qi * NB + 128 * c + 128],
                                i128,
                            )
                        wt = wt_pool.tile([128, STRIP], FP32, tag="wt")
                        if c < 3:
                            nc.scalar.copy(out=wt[:, :F], in_=tin[:, :F])
                        else:
                            nc.vector.tensor_copy(out=wt[:, :F], in_=tin[:, :F])
                        wts.append(wt)
                    # --- matmul: src.T strip (64, F), accumulate over 4 n-chunks
                    psS = psrc_pool.tile([64, STRIP], FP32, tag="psrc")
                    for c in range(4):
                        nc.tensor.matmul(
                            psS[:, :F],
                            bT_sb[:, L * c : L * c + L],
                            wts[c][:, :F],
                            start=(c == 0),
                            stop=(c == 3),
                        )
                    # --- overlap-add: oaa[i, j] = A[f0+j, i] + B[f0+j-1, i]
                    # tensor_tensor can't read both inputs from PSUM, so stage
                    # the B-half (source[:, 32:64].T) into SBUF via ACT first.
                    cpB = oaa_pool.tile([32, STRIP], FP32, tag="cpB")
                    nc.scalar.copy(out=cpB[:, :F], in_=psS[32:64, :F])
                    oaa = oaa_pool.tile([32, STRIP], FP32, tag="oaa")
                    nc.vector.tensor_add(
                        out=oaa[:, 1:F], in0=psS[0:32, 1:F], in1=cpB[:, 0 : F - 1]
                    )
                    if si == 0:
                        nc.vector.tensor_copy(out=oaa[:, 0:1], in_=psS[0:32, 0:1])
                    else:
                        nc.vector.tensor_add(
                            out=oaa[:, 0:1],
                            in0=psS[0:32, 0:1],
                            in1=prev_cpB[:, prev_F - 1 : prev_F],
                        )
                    # --- transpose out: (32, F) -> q x (128, 32), rows j on partitions
                    ptout = ptout_pool.tile([128, (STRIP // 128) * 32], FP32, tag="ptout")
                    for qi in range(q):
                        nc.tensor.transpose(
                            ptout[:, 32 * qi : 32 * qi + 32],
                            oaa[:, 128 * qi : 128 * qi + 128],
                            i32,
                        )
                    osb = osb_pool.tile([128, (STRIP // 128) * 32], FP32, tag="osb")
                    nc.vector.tensor_copy(out=osb[:, : 32 * q], in_=ptout[:, : 32 * q])
                    nc.sync.dma_start(
                        out=out[f0 : f0 + F, :].rearrange("(q p) i -> p q i", p=128),
                        in_=osb[:, : 32 * q].rearrange("p (q i) -> p q i", i=32),
                    )
                    prev_cpB, prev_F = cpB, F
                # --- final subframe j=frames: B-half of the last frame
                fin = osb_pool.tile([32, 1], FP32, tag="fin")
                nc.vector.tensor_copy(
                    out=fin, in_=prev_cpB[:, prev_F - 1 : prev_F]
                )
                nc.sync.dma_start(
                    out=out[frames : frames + 1, :].rearrange("a i -> i a"), in_=fin
                )
    nc.finalize()
    return nc


def _in_maps(weight, bT, n_cores, frames):
    id128 = np.eye(128, dtype=np.float32)
    id32 = np.eye(32, dtype=np.float32)
    return [
        {
            "w": np.ascontiguousarray(weight[c, :frames]),
            "bT": bT,
            "id128": id128,
            "id32": id32,
        }
        for c in range(n_cores)
    ]


def kernel(weight, basis_signal_weight):
    weight = np.ascontiguousarray(np.asarray(weight, dtype=np.float32))
    basis = np.asarray(basis_signal_weight, dtype=np.float32)
    bT = np.ascontiguousarray(basis.T)  # (512, 64)
    nc = build_nc()
    res = run_bass_kernel_spmd(
        nc, _in_maps(weight, bT, BATCH, FRAMES), core_ids=list(range(BATCH))
    )
    return np.stack([r["out"].reshape(-1) for r in res.results])


# revision 14
# speedup vs baseline: 2.6855x; 2.6855x over previous
"""Trainium2 Bass kernel for BasisSignalLayer (matmul + 50%-overlap-add).

Reference computation:
    source = einsum("bkn,ln->bkl", weight, basis_signal_weight)   # (B, K, L)
    out    = overlap_and_add(source, L // 2)                       # (B, 32*(K-1)+64)

With L=64 and frame_step=32 (gcd trick in the reference), the scatter-add
reduces to: output subframe j (32 floats) = source[j, 0:32] + source[j-1, 32:64],
with j in [0, K] (source[-1] = source[K] = 0 contributions at the edges).

Per-core dataflow (batch-parallel across 8 cores, one batch element each):
  - DMA natural-layout weight strips (512 frames x 512 basis) into SBUF
  - PE transposes 128x128 tiles to put the contraction dim (n) on partitions
  - ACT/DVE copy transposed tiles PSUM -> SBUF
  - 4 accumulating matmuls: psum(64, F) = basisT.T @ wT  (source.T strip)
  - DVE shifted add: oaa[i, j] = psum[i, j] + psum[32+i, j-1]  (the overlap-add)
  - PE transposes (32,128)->(128,32) to get output rows on partitions
  - DVE copy + DMA out
"""

import numpy as np

import concourse.bacc as bacc
import concourse.bass as bass
import concourse.mybir as mybir
from concourse import tile
from concourse.bass_utils import run_bass_kernel_spmd

FRAMES = 16000
NB = 512  # basis count (contraction dim)
L = 64  # frame length
BATCH = 8
STRIP = 512  # frames per strip
FP32 = mybir.dt.float32


def _strips(frames):
    out, f0 = [], 0
    while f0 < frames:
        F = min(STRIP, frames - f0)
        assert F % 128 == 0
        out.append((f0, F))
        f0 += F
    return out


def build_nc(frames=FRAMES, repeat=1, skip=()):
    """Build the single-core Bass program (SPMD: same program on all cores).

    skip: diagnostic-only ablations ("tin" = no PE input transposes,
    "mm" = no matmuls/OAA/output path). Results are wrong with any skip;
    used to attribute time between engines since no NTFF profiling exists.
    """
    nc = bacc.Bacc()
    w = nc.dram_tensor("w", [frames, NB], FP32, kind="ExternalInput")
    bT = nc.dram_tensor("bT", [NB, L], FP32, kind="ExternalInput")
    id128 = nc.dram_tensor("id128", [128, 128], FP32, kind="ExternalInput")
    id32 = nc.dram_tensor("id32", [32, 32], FP32, kind="ExternalInput")
    nsub = frames + 1
    out = nc.dram_tensor("out", [nsub, 32], FP32, kind="ExternalOutput")

    strips = _strips(frames)

    with tile.TileContext(nc) as tc:
        with (
            tc.tile_pool(name="consts", bufs=1) as consts,
            tc.tile_pool(name="wn", bufs=3) as wn_pool,
            tc.tile_pool(name="wt", bufs=8) as wt_pool,
            tc.tile_pool(name="oaa", bufs=3) as oaa_pool,
            tc.tile_pool(name="osb", bufs=3) as osb_pool,
            tc.tile_pool(name="ptin", bufs=4, space="PSUM") as ptin_pool,
            tc.tile_pool(name="psrc", bufs=2, space="PSUM") as psrc_pool,
            tc.tile_pool(name="ptout", bufs=1, space="PSUM") as ptout_pool,
        ):
            bT_sb = consts.tile([128, 4 * L], FP32)
            for c in range(4):
                nc.sync.dma_start(
                    out=bT_sb[:, L * c : L * c + L], in_=bT[128 * c : 128 * c + 128, :]
                )
            i128 = consts.tile([128, 128], FP32)
            nc.sync.dma_start(out=i128, in_=id128[:, :])
            i32 = consts.tile([32, 32], FP32)
            nc.sync.dma_start(out=i32, in_=id32[:, :])

            for _rep in range(repeat):
                prev_src = None
                prev_F = None
                for si, (f0, F) in enumerate(strips):
                    q = F // 128
                    # --- load natural strip: (128, q*512), col = qi*512 + n
                    wn = wn_pool.tile([128, (STRIP // 128) * NB], FP32, tag="wn")
                    nc.sync.dma_start(
                        out=wn[:, : q * NB].rearrange("p (q n) -> p q n", n=NB),
                        in_=w[f0 : f0 + F, :].rearrange("(q p) n -> p q n", p=128),
                    )
                    # --- transpose to (n, f) chunks + copy PSUM->SBUF
                    wts = []
                    if "tin" in skip:
                        # diagnostic: fake wT via direct (untransposed) copies
                        for c in range(4):
                            wt = wt_pool.tile([128, STRIP], FP32, tag="wt")
                            if c < 3:
                                nc.scalar.copy(out=wt[:, :F], in_=wn[:, :F])
                            else:
                                nc.vector.tensor_copy(out=wt[:, :F], in_=wn[:, :F])
                            wts.append(wt)
                    else:
                        for c in range(4):
                            tin = ptin_pool.tile([128, STRIP], FP32, tag="ptin")
                            for qi in range(q):
                                nc.tensor.transpose(
                                    tin[:, 128 * qi : 128 * qi + 128],
                                    wn[:, qi * NB + 128 * c : qi * NB + 128 * c + 128],
                                    i128,
                                )
                            wt = wt_pool.tile([128, STRIP], FP32, tag="wt")
                            if c < 3:
                                nc.scalar.copy(out=wt[:, :F], in_=tin[:, :F])
                            else:
                                nc.vector.tensor_copy(out=wt[:, :F], in_=tin[:, :F])
                            wts.append(wt)
                    if "mm" in skip:
                        # diagnostic: just DMA out a slice of wT to keep the
                        # pipeline (and output traffic) alive
                        osb0 = osb_pool.tile([128, (STRIP // 128) * 32], FP32, tag="osb")
                        nc.vector.tensor_copy(out=osb0[:, : 32 * q], in_=wts[0][:, : 32 * q])
                        nc.sync.dma_start(
                            out=out[f0 : f0 + F, :].rearrange("(q p) i -> p q i", p=128),
                            in_=osb0[:, : 32 * q].rearrange("p (q i) -> p q i", i=32),
                        )
                        prev_cpB, prev_F = None, F
                        continue
                    # --- matmul: src.T strip (64, F), accumulate over 4 n-chunks
                    psS = psrc_pool.tile([64, STRIP], FP32, tag="psrc")
                    for c in range(4):
                        nc.tensor.matmul(
                            psS[:, :F],
                            bT_sb[:, L * c : L * c + L],
                            wts[c][:, :F],
                            start=(c == 0),
                            stop=(c == 3),
                        )
                    # --- overlap-add: oaa[i, j] = A[f0+j, i] + B[f0+j-1, i]
                    # tensor_tensor can't read both inputs from PSUM, so stage
                    # the B-half (source[:, 32:64].T) into SBUF via ACT first.
                    cpB = oaa_pool.tile([32, STRIP], FP32, tag="cpB")
                    nc.scalar.copy(out=cpB[:, :F], in_=psS[32:64, :F])
                    oaa = oaa_pool.tile([32, STRIP], FP32, tag="oaa")
                    nc.vector.tensor_add(
                        out=oaa[:, 1:F], in0=psS[0:32, 1:F], in1=cpB[:, 0 : F - 1]
                    )
                    if si == 0:
                        nc.vector.tensor_copy(out=oaa[:, 0:1], in_=psS[0:32, 0:1])
                    else:
                        nc.vector.tensor_add(
                            out=oaa[:, 0:1],
                            in0=psS[0:32, 0:1],
                            in1=prev_cpB[:, prev_F - 1 : prev_F],
                        )
                    # --- transpose out: (32, F) -> q x (128, 32), rows j on partitions
                    ptout = ptout_pool.tile([128, (STRIP // 128) * 32], FP32, tag="ptout")
                    for qi in range(q):
                        nc.tensor.transpose(
                            ptout[:, 32 * qi : 32 * qi + 32],
                            oaa[:, 128 * qi : 128 * qi + 128],
                            i32,
                        )
                    osb = osb_pool.tile([128, (STRIP // 128) * 32], FP32, tag="osb")
                    nc.vector.tensor_copy(out=osb[:, : 32 * q], in_=ptout[:, : 32 * q])
                    nc.sync.dma_start(
                        out=out[f0 : f0 + F, :].rearrange("(q p) i -> p q i", p=128),
                        in_=osb[:, : 32 * q].rearrange("p (q i) -> p q i", i=32),
                    )
                    prev_cpB, prev_F = cpB, F
                # --- final subframe j=frames: B-half of the last frame
                if "mm" not in skip:
                    fin = osb_pool.tile([32, 1], FP32, tag="fin")
                    nc.vector.tensor_copy(
                        out=fin, in_=prev_cpB[:, prev_F - 1 : prev_F]
                    )
                    nc.sync.dma_start(
                        out=out[frames : frames + 1, :].rearrange("a i -> i a"),
                        in_=fin,
                    )
    nc.finalize()
    return nc


def _in_maps(weight, bT, n_cores, frames):
    id128 = np.eye(128, dtype=np.float32)
    id32 = np.eye(32, dtype=np.float32)
    return [
        {
            "w": np.ascontiguousarray(weight[c, :frames]),
            "bT": bT,
            "id128": id128,
            "id32": id32,
        }
        for c in range(n_cores)
    ]


def kernel(weight, basis_signal_weight):
    weight = np.ascontiguousarray(np.asarray(weight, dtype=np.float32))
    basis = np.asarray(basis_signal_weight, dtype=np.float32)
    bT = np.ascontiguousarray(basis.T)  # (512, 64)
    nc = build_nc()
    res = run_bass_kernel_spmd(
        nc, _in_maps(weight, bT, BATCH, FRAMES), core_ids=list(range(BATCH))
    )
    return np.stack([r["out"].reshape(-1) for r in res.results])


# revision 26
# speedup vs baseline: 3.5089x; 1.3066x over previous
"""Trainium2 Bass kernel for BasisSignalLayer (matmul + 50%-overlap-add).

Reference computation:
    source = einsum("bkn,ln->bkl", weight, basis_signal_weight)   # (B, K, L)
    out    = overlap_and_add(source, L // 2)                       # (B, 32*(K-1)+64)

With L=64 and frame_step=32 (gcd trick in the reference), the scatter-add
reduces to: output subframe j (32 floats) = source[j, 0:32] + source[j-1, 32:64],
with j in [0, K] (source[-1] = source[K] = 0 contributions at the edges).

Per-core dataflow (batch-parallel across 8 cores, one batch element each):
  - DMA natural-layout weight strips (512 frames x 512 basis) into SBUF
  - PE transposes 128x128 tiles to put the contraction dim (n) on partitions
  - ACT/DVE copy transposed tiles PSUM -> SBUF
  - 4 accumulating matmuls: psum(64, F) = basisT.T @ wT  (source.T strip)
  - DVE shifted add: oaa[i, j] = psum[i, j] + psum[32+i, j-1]  (the overlap-add)
  - PE transposes (32,128)->(128,32) to get output rows on partitions
  - DVE copy + DMA out
"""

import numpy as np

import concourse.bacc as bacc
import concourse.bass as bass
import concourse.mybir as mybir
from concourse import tile
from concourse.bass_utils import run_bass_kernel_spmd

FRAMES = 16000
NB = 512  # basis count (contraction dim)
L = 64  # frame length
BATCH = 8
STRIP = 512  # frames per strip
FP32 = mybir.dt.float32


def _strips(frames):
    out, f0 = [], 0
    while f0 < frames:
        F = min(STRIP, frames - f0)
        assert F % 128 == 0
        out.append((f0, F))
        f0 += F
    return out


def build_nc(frames=FRAMES, repeat=1, skip=(), warm=False, rdt=False):
    """Build the single-core Bass program (SPMD: same program on all cores).

    skip: diagnostic-only ablations ("tin" = no PE input transposes,
    "mm" = no matmuls/OAA/output path). Results are wrong with any skip;
    used to attribute time between engines since no NTFF profiling exists.

    rdt: use float32r (same 4-byte layout, faster PE path: 1 cy/row matmul
    vs 4, 1.5 cy/row transpose vs 2) for the weight/basis operand tensors.
    """
    WDT = mybir.dt.float32r if rdt else FP32
    nc = bacc.Bacc()
    w = nc.dram_tensor("w", [frames, NB], WDT, kind="ExternalInput")
    bT = nc.dram_tensor("bT", [NB, L], WDT, kind="ExternalInput")
    id128 = nc.dram_tensor("id128", [128, 128], WDT, kind="ExternalInput")
    id32 = nc.dram_tensor("id32", [32, 32], FP32, kind="ExternalInput")
    nsub = frames + 1
    out = nc.dram_tensor("out", [nsub, 32], FP32, kind="ExternalOutput")
    dbg = nc.dram_tensor("dbg", [32, 1], FP32, kind="ExternalOutput") if warm else None

    strips = _strips(frames)

    with tile.TileContext(nc) as tc:
        with (
            tc.tile_pool(name="consts", bufs=1) as consts,
            tc.tile_pool(name="wn", bufs=3) as wn_pool,
            tc.tile_pool(name="wt", bufs=8) as wt_pool,
            tc.tile_pool(name="oaa", bufs=3) as oaa_pool,
            tc.tile_pool(name="osb", bufs=3) as osb_pool,
            tc.tile_pool(name="ptin", bufs=4, space="PSUM") as ptin_pool,
            tc.tile_pool(name="psrc", bufs=2, space="PSUM") as psrc_pool,
            tc.tile_pool(name="ptout", bufs=1, space="PSUM") as ptout_pool,
            tc.tile_pool(name="pwarm", bufs=1, space="PSUM") as pwarm_pool,
        ):
            bT_sb = consts.tile([128, 4 * L], WDT)
            for c in range(4):
                nc.sync.dma_start(
                    out=bT_sb[:, L * c : L * c + L], in_=bT[128 * c : 128 * c + 128, :]
                )
            i128 = consts.tile([128, 128], WDT)
            nc.sync.dma_start(out=i128, in_=id128[:, :])
            i32 = consts.tile([32, 32], FP32)
            nc.sync.dma_start(out=i32, in_=id32[:, :])

            dwarm = None
            if warm:
                # HAM warm-up: transpose-mode PE work doesn't register as
                # "busy" for the clock-gate monitor, so sustain ~4.5us of real
                # matmuls once, then sprinkle tiny ones to keep MID non-idle.
                dwarm = pwarm_pool.tile([128, 512], FP32)
                for _ in range(40):
                    nc.tensor.matmul(dwarm[:, 0:128], i128, i128, start=True, stop=True)

            for _rep in range(repeat):
                prev_cpB = None
                prev_F = None
                for si, (f0, F) in enumerate(strips):
                    q = F // 128
                    # --- load natural strip: (128, q*512), col = qi*512 + n
                    wn = wn_pool.tile([128, (STRIP // 128) * NB], WDT, tag="wn")
                    nc.sync.dma_start(
                        out=wn[:, : q * NB].rearrange("p (q n) -> p q n", n=NB),
                        in_=w[f0 : f0 + F, :].rearrange("(q p) n -> p q n", p=128),
                    )
                    # --- transpose to (n, f) chunks + copy PSUM->SBUF
                    wts = []
                    if "tin" in skip:
                        # diagnostic: fake wT via direct (untransposed) copies
                        if "cp" not in skip:
                            for c in range(4):
                                wt = wt_pool.tile([128, STRIP], WDT, tag="wt")
                                if c < 3:
                                    nc.scalar.copy(out=wt[:, :F], in_=wn[:, :F])
                                else:
                                    nc.vector.tensor_copy(out=wt[:, :F], in_=wn[:, :F])
                                wts.append(wt)
                    else:
                        for c in range(4):
                            tin = ptin_pool.tile([128, STRIP], WDT, tag="ptin")
                            for qi in range(q):
                                nc.tensor.transpose(
                                    tin[:, 128 * qi : 128 * qi + 128],
                                    wn[:, qi * NB + 128 * c : qi * NB + 128 * c + 128],
                                    i128,
                                )
                            wt = wt_pool.tile([128, STRIP], WDT, tag="wt")
                            if c < 3:
                                nc.scalar.copy(out=wt[:, :F], in_=tin[:, :F])
                            else:
                                nc.vector.tensor_copy(out=wt[:, :F], in_=tin[:, :F])
                            wts.append(wt)
                            if warm and c == 1:
                                nc.tensor.matmul(
                                    dwarm[:, 128:160],
                                    i128,
                                    i128[:, 0:32],
                                    start=True,
                                    stop=True,
                                )
                    if "mm" in skip:
                        # diagnostic: DMA out from wT (or wn for pure-DMA) to
                        # keep pipeline + output traffic alive
                        src_t = wn if "cp" in skip else wts[0]
                        nc.sync.dma_start(
                            out=out[f0 : f0 + F, :].rearrange("(q p) i -> p q i", p=128),
                            in_=src_t[:, : 32 * q].rearrange("p (q i) -> p q i", i=32),
                        )
                        prev_cpB, prev_F = None, F
                        continue
                    # --- matmul: src.T strip (64, F), accumulate over 4 n-chunks
                    psS = psrc_pool.tile([64, STRIP], FP32, tag="psrc")
                    for c in range(4):
                        nc.tensor.matmul(
                            psS[:, :F],
                            bT_sb[:, L * c : L * c + L],
                            wts[c][:, :F],
                            start=(c == 0),
                            stop=(c == 3),
                        )
                    # --- overlap-add: oaa[i, j] = A[f0+j, i] + B[f0+j-1, i]
                    # tensor_tensor can't read both inputs from PSUM, so stage
                    # the B-half (source[:, 32:64].T) into SBUF via ACT first.
                    cpB = oaa_pool.tile([32, STRIP], FP32, tag="cpB")
                    nc.scalar.copy(out=cpB[:, :F], in_=psS[32:64, :F])
                    oaa = oaa_pool.tile([32, STRIP], FP32, tag="oaa")
                    nc.vector.tensor_add(
                        out=oaa[:, 1:F], in0=psS[0:32, 1:F], in1=cpB[:, 0 : F - 1]
                    )
                    if si == 0:
                        nc.vector.tensor_copy(out=oaa[:, 0:1], in_=psS[0:32, 0:1])
                    else:
                        nc.vector.tensor_add(
                            out=oaa[:, 0:1],
                            in0=psS[0:32, 0:1],
                            in1=prev_cpB[:, prev_F - 1 : prev_F],
                        )
                    # --- transpose out: (32, F) -> q x (128, 32), rows j on partitions
                    ptout = ptout_pool.tile([128, (STRIP // 128) * 32], FP32, tag="ptout")
                    for qi in range(q):
                        nc.tensor.transpose(
                            ptout[:, 32 * qi : 32 * qi + 32],
                            oaa[:, 128 * qi : 128 * qi + 128],
                            i32,
                        )
                    osb = osb_pool.tile([128, (STRIP // 128) * 32], FP32, tag="osb")
                    nc.vector.tensor_copy(out=osb[:, : 32 * q], in_=ptout[:, : 32 * q])
                    nc.sync.dma_start(
                        out=out[f0 : f0 + F, :].rearrange("(q p) i -> p q i", p=128),
                        in_=osb[:, : 32 * q].rearrange("p (q i) -> p q i", i=32),
                    )
                    prev_cpB, prev_F = cpB, F
                # --- final subframe j=frames: B-half of the last frame
                if "mm" not in skip:
                    fin = osb_pool.tile([32, 1], FP32, tag="fin")
                    nc.vector.tensor_copy(
                        out=fin, in_=prev_cpB[:, prev_F - 1 : prev_F]
                    )
                    nc.sync.dma_start(
                        out=out[frames : frames + 1, :].rearrange("a i -> i a"),
                        in_=fin,
                    )
            if warm:
                # consume dwarm so the warm-up matmuls can't be dead-code'd
                dsb = osb_pool.tile([32, 1], FP32, tag="dsb")
                nc.vector.tensor_copy(out=dsb, in_=dwarm[0:32, 0:1])
                nc.sync.dma_start(out=dbg[:, :], in_=dsb)
    nc.finalize()
    return nc


def _in_maps(weight, bT, n_cores, frames):
    id128 = np.eye(128, dtype=np.float32)
    id32 = np.eye(32, dtype=np.float32)
    return [
        {
            "w": np.ascontiguousarray(weight[c, :frames]),
            "bT": bT,
            "id128": id128,
            "id32": id32,
        }
        for c in range(n_cores)
    ]


def kernel(weight, basis_signal_weight):
    weight = np.ascontiguousarray(np.asarray(weight, dtype=np.float32))
    basis = np.asarray(basis_signal_weight, dtype=np.float32)
    bT = np.ascontiguousarray(basis.T)  # (512, 64)
    nc = build_nc()
    res = run_bass_kernel_spmd(
        nc, _in_maps(weight, bT, BATCH, FRAMES), core_ids=list(range(BATCH))
    )
    return np.stack([r["out"].reshape(-1) for r in res.results])
